# revision 16
# baseline (speedup 1.0000x reference)
"""Trainium2 Bass kernel for nn_CRF (loopy belief propagation / CRF message passing).

Pure data-parallel: batch dim B=64 is sharded 8 ways across the 8 NeuronCores
(8 batches per core). Inside one core, all 8 local batches are processed as two
"fat tile" groups of 4 batches with free-dim layout (y, k, b) — batch
*innermost* — so that every big elementwise op (including the belief broadcast)
streams in the DVE's 2x bf16 mode as one instruction.

Numerical design (validated against the float32 reference to 0 abs error over
many seeds in simulation):
  * all big tensors are bf16 — the LBP consensus dynamics have enormous vote
    margins, so bf16 reproduces the exact reference output,
  * messages are stored as *reciprocals* of the transposed messages (M2r), so
    the per-iteration division becomes a multiply,
  * reciprocals use a one-instruction exponent-flip approximation: inputs are
    pre-scaled by C_RECIP (folded into existing ScalarE scale slots), then a
    single bitwise XOR with 0x7FFF on the bf16 bit pattern yields ~1/x with
    NO overshoot (max undershoot ~11%). No-overshoot keeps messages < 1 so
    the 128-way factor product cannot exceed float32 range.
  * the product over neighbors uses pairwise bf16 multiply trees; the final
    f32 "inter" is clamped to 3.3e38 to neutralize the 2^128 corner case.
"""

import sys

sys.path.insert(0, "/opt/trn_rl_repo")

import numpy as np

B, N, D, Y = 64, 128, 128, 16
NCORES = 8
BL = B // NCORES          # batches per core
G = 4                     # fat-tile groups per core
BG = BL // G              # batches per group
NSUP = 80                 # num_supports (hardcoded per problem spec)
ITERS = 7                 # lbp_count - 1
C_RECIP = 4.48542355      # reciprocal pre-scale (XOR 0x7FFF ~ magic 0x7EEA)
INTER_CLAMP = 3.3e38      # keep inter finite in f32

_cache = {}


def _ap(base, free_dims):
    """Build an AP on base's tensor with explicit free-dim [step, count]
    entries; the partition dim entry is inherited from base (its step is the
    tile's flat row pitch, not 1)."""
    import concourse.bass as bass

    return bass.AP(tensor=base.tensor, offset=base.offset,
                   ap=[list(base.ap[0])] + [list(d) for d in free_dims])


def build_program():
    import concourse.bass as bass
    import concourse.tile as tile
    from concourse import bacc, mybir
    from concourse.masks import make_identity

    dt = mybir.dt
    F32, BF16, I16 = dt.float32, dt.bfloat16, dt.int16
    AX = mybir.AxisListType
    OP = mybir.AluOpType
    ACTF = mybir.ActivationFunctionType

    nc = bacc.Bacc(None, target_bir_lowering=False)

    inp_d = nc.dram_tensor("inp_data", [BL, N, D], F32, kind="ExternalInput")
    una_d = nc.dram_tensor("unary_comp", [BL, N, Y], F32, kind="ExternalInput")
    bin_d = nc.dram_tensor("binary_comp", [BL, N, N], F32, kind="ExternalInput")
    aff_d = nc.dram_tensor("affinity_mat", [BL, N, N], F32, kind="ExternalInput")
    out_d = nc.dram_tensor("out", [BL, N, N], F32, kind="ExternalOutput")

    with tile.TileContext(nc) as tc:
        import contextlib
        ctx = contextlib.ExitStack()
        with ctx:
            singles = ctx.enter_context(tc.tile_pool(name="singles", bufs=1))
            stage = ctx.enter_context(tc.tile_pool(name="stage", bufs=3))
            smalls = ctx.enter_context(tc.tile_pool(name="smalls", bufs=4))
            work = ctx.enter_context(tc.tile_pool(name="work", bufs=4))
            tree = ctx.enter_context(tc.tile_pool(name="tree", bufs=2))
            belp = ctx.enter_context(tc.tile_pool(name="belp", bufs=3))
            outp = ctx.enter_context(tc.tile_pool(name="outp", bufs=2))
            psum = ctx.enter_context(tc.tile_pool(name="psum", bufs=3, space="PSUM"))

            identity = singles.tile([N, N], BF16)
            make_identity(nc, identity)

            # ---------- persistent per-group tensors ----------
            m2r = [singles.tile([N, Y, N, BG], BF16, tag=f"m2r{g}",
                                name=f"m2r{g}") for g in range(G)]
            binb = [singles.tile([N, N, BG], BF16, tag=f"bin{g}",
                                 name=f"bin{g}") for g in range(G)]
            maskb = [singles.tile([N, N, BG], BF16, tag=f"mt{g}",
                                  name=f"maskb{g}") for g in range(G)]
            ue = [singles.tile([N, Y, BG], BF16, tag=f"ue{g}",
                               name=f"ue{g}") for g in range(G)]
            bel = [None] * G

            for g in range(G):
                nc.vector.memset(ue[g][:], 1.0)

            # ---------- setup: loads, mask, unary, initial belief ----------
            for g in range(G):
                belt = belp.tile([N, Y, BG], BF16, tag="bel")
                bel[g] = belt
                for bg in range(BG):
                    b = g * BG + bg
                    # --- initial belief from cosine similarity ---
                    st = stage.tile([N, D], F32, tag="st")
                    nc.sync.dma_start(out=st[:], in_=inp_d[b, :, :])
                    sq = smalls.tile([N, D], F32, tag="sq")
                    ss = smalls.tile([N, 1], F32, tag="ss")
                    nc.scalar.activation(sq[:], st[:], ACTF.Square, accum_out=ss[:])
                    nrmn = smalls.tile([N, 1], F32, tag="nrmn")
                    nc.scalar.activation(nrmn[:], ss[:], ACTF.Sqrt)
                    nc.vector.tensor_scalar_max(nrmn[:], nrmn[:], 1e-8)
                    rsn = smalls.tile([N, 1], F32, tag="rsn")
                    nc.vector.reciprocal(rsn[:], nrmn[:])
                    nrmb = smalls.tile([N, D], BF16, tag="nrmb")
                    nc.vector.tensor_scalar_mul(nrmb[:], st[:], rsn[:])
                    ps_t = psum.tile([N, D], BF16, tag="ps_small", name="ps_t", bufs=1)
                    nc.tensor.transpose(ps_t[:], nrmb[:], identity)
                    nrmT = smalls.tile([N, D], BF16, tag="nrmT")
                    nc.scalar.copy(nrmT[:], ps_t[:])
                    gps = psum.tile([N, Y], F32, tag="ps_small", name="gps", bufs=1)
                    nc.tensor.matmul(gps[:], nrmT[:], nrmT[:, 0:Y])
                    nmax = smalls.tile([N, 1], F32, tag="nmax")
                    nc.vector.tensor_reduce(nmax[:], gps[:], axis=AX.X, op=OP.max,
                                            negate=True)
                    e0 = smalls.tile([N, Y], BF16, tag="e0")
                    s0 = smalls.tile([N, 1], F32, tag="s0")
                    nc.scalar.activation(e0[:], gps[:], ACTF.Exp, bias=nmax[:],
                                         accum_out=s0[:])
                    rs0 = smalls.tile([N, 1], F32, tag="rs0")
                    nc.vector.reciprocal(rs0[:], s0[:])
                    nc.vector.tensor_scalar_mul(belt[:, :, bg], e0[:], rs0[:])

                    # --- binary -> bf16 (k, b layout) ---
                    st2 = stage.tile([N, N], F32, tag="st2")
                    nc.sync.dma_start(out=st2[:], in_=bin_d[b, :, :])
                    nc.vector.tensor_copy(binb[g][:, :, bg], st2[:])

                    # --- mask = (affinity > 0.001), transposed ---
                    st3 = stage.tile([N, N], F32, tag="st3")
                    nc.sync.dma_start(out=st3[:], in_=aff_d[b, :, :])
                    nc.vector.tensor_scalar(maskb[g][:, :, bg], st3[:], 0.001,
                                            None, op0=OP.is_gt)

                    # --- unary_eff (rows >= NSUP are 1.0; partition starts
                    # must be 32-aligned, so memset-all then copy 0:64, 64:80) ---
                    st4 = stage.tile([N, Y], F32, tag="st4")
                    nc.sync.dma_start(out=st4[:], in_=una_d[b, :, :])
                    nc.vector.tensor_copy(ue[g][0:64, :, bg], st4[0:64, :])
                    nc.vector.tensor_copy(ue[g][64:NSUP, :, bg], st4[64:NSUP, :])

            # ---------- broadcast-read AP helpers ----------
            def bc_bin(g):      # binary[j,k] -> (y,k,b)
                t = binb[g][:]
                return _ap(t, [[0, Y], [BG, N], [1, BG]])

            def bc_mask(g):
                t = maskb[g][:]
                return _ap(t, [[0, Y], [BG, N], [1, BG]])

            def bc_bel(g):      # bel[j,y] -> (y,k,b)
                t = bel[g][:]
                return _ap(t, [[BG, Y], [0, N], [1, BG]])

            def bc_overy(t, cnt_k):   # rd[j,(k,b)] -> (y,k,b)
                a = t[:]
                return _ap(a, [[0, Y], [BG, cnt_k], [1, BG]])

            def perm_by(t):     # [.., Y, BG] tile read as (b, y) for reduce over y
                a = t[:]
                return _ap(a, [[1, BG], [BG, Y]])

            def bc_overy_small(t):  # [N, BG] -> (y, b)
                a = t[:]
                return _ap(a, [[0, Y], [1, BG]])

            # ---------- LBP iterations ----------
            for it in range(ITERS):
                qs = []
                for g in range(G):
                    H = work.tile([N, Y, N, BG], BF16, tag="hfr")
                    if it == 0:
                        # msg0 is uniform 1/Y -> M2r0 = 16 exactly; fold into a
                        # faster single-input op (4x mode) and skip the memsets
                        nc.vector.tensor_scalar_mul(H[:], bc_bin(g), 16.0)
                    else:
                        nc.vector.tensor_tensor(out=H[:], in0=m2r[g][:],
                                                in1=bc_bin(g), op=OP.mult)
                    q = work.tile([N, Y, N, BG], BF16, tag="qfac")
                    nc.vector.tensor_tensor(out=q[:], in0=H[:], in1=bc_bel(g),
                                            op=OP.mult)
                    qs.append(q)

                rds, m1ns = [], []
                for g in range(G):
                    q = qs[g]
                    denp = psum.tile([N, N, BG], F32, tag="denp", name="denp",
                                     bufs=2)
                    for y in range(Y):
                        nc.tensor.matmul(denp[:], identity[:], q[:, y, :, :],
                                         start=(y == 0), stop=(y == Y - 1))
                    den2 = smalls.tile([N, N, BG], BF16, tag="den2")
                    nc.vector.tensor_scalar(den2[:], denp[:], C_RECIP,
                                            16e-4 * C_RECIP,
                                            op0=OP.mult, op1=OP.add)
                    rd = smalls.tile([N, N, BG], BF16, tag="rd")
                    nc.vector.tensor_scalar(rd[:].bitcast(I16),
                                            den2[:].bitcast(I16),
                                            0x7FFF, None, op0=OP.bitwise_xor)
                    rds.append(rd)

                    qe = tree.tile([N, Y, N, BG], BF16, tag="scratch2")
                    nc.vector.tensor_scalar_add(qe[:], q[:], 1e-4)
                    m1n = work.tile([N, Y, N, BG], BF16, tag="m1n", bufs=3)
                    nc.vector.tensor_tensor(out=m1n[:], in0=qe[:],
                                            in1=bc_overy(rd, N), op=OP.mult)
                    m1ns.append(m1n)

                # transpose msg_new (per batch) into m2n, then m2r = recip(m2n)
                m2ns = []
                for g in range(G):
                    m1n = m1ns[g]
                    m2n = work.tile([N, Y, N, BG], BF16, tag="m2n")
                    for bg in range(BG):
                        pst = psum.tile([N, Y, N], BF16, tag="pst", bufs=2)
                        for y in range(Y):
                            nc.tensor.transpose(pst[:, y, :],
                                                m1n[:, y, :, bg], identity)
                        nc.scalar.activation(m2n[:, :, :, bg], pst[:],
                                             ACTF.Copy, scale=C_RECIP)
                    if it < ITERS - 1:
                        nc.vector.tensor_scalar(m2r[g][:].bitcast(I16),
                                                m2n[:].bitcast(I16),
                                                0x7FFF, None, op0=OP.bitwise_xor)
                    m2ns.append(m2n)

                # belief update: factor[j,k,y] = 1 + mask[j,k]*msg_new[k,j,y]
                # msg_new[k,j,y] in [j,(y,k,b)] layout is exactly m2n
                for g in range(G):
                    fr = work.tile([N, Y, N, BG], BF16, tag="hfr")
                    nc.vector.tensor_tensor(out=fr[:], in0=m2ns[g][:],
                                            in1=bc_mask(g), op=OP.mult)
                    fac = work.tile([N, Y, N, BG], BF16, tag="qfac")
                    nc.vector.tensor_scalar(fac[:], fr[:], 1.0 / C_RECIP, 1.0,
                                            op0=OP.mult, op1=OP.add)
                    p = fac
                    cnt = N
                    while cnt > 2:
                        h = cnt // 2
                        pn = tree.tile([N, Y, h, BG], BF16, tag="scratch")
                        nc.vector.tensor_tensor(out=pn[:], in0=p[:, :, 0:h, :],
                                                in1=p[:, :, h:cnt, :], op=OP.mult)
                        p = pn
                        cnt = h
                    inter = smalls.tile([N, Y, BG], F32, tag="inter")
                    nc.vector.tensor_tensor(out=inter[:], in0=p[:, :, 0, :],
                                            in1=p[:, :, 1, :], op=OP.mult)
                    nc.vector.tensor_scalar(inter[:], inter[:], INTER_CLAMP, None,
                                            op0=OP.min)
                    nc.vector.tensor_tensor(out=inter[:], in0=inter[:],
                                            in1=ue[g][:], op=OP.mult)
                    nm = smalls.tile([N, BG], F32, tag="nm")
                    nc.vector.tensor_reduce(nm[:], perm_by(inter), axis=AX.X,
                                            op=OP.max, negate=True)
                    dd = smalls.tile([N, Y, BG], F32, tag="dd")
                    nc.vector.tensor_tensor(out=dd[:], in0=inter[:],
                                            in1=bc_overy_small(nm), op=OP.add)
                    ee = smalls.tile([N, Y, BG], BF16, tag="ee")
                    nc.scalar.activation(ee[:], dd[:], ACTF.Exp)
                    sm = smalls.tile([N, BG], F32, tag="sm")
                    nc.vector.tensor_reduce(sm[:], perm_by(ee), axis=AX.X,
                                            op=OP.add)
                    rsm = smalls.tile([N, BG], F32, tag="rsm")
                    nc.vector.reciprocal(rsm[:], sm[:])
                    belt = belp.tile([N, Y, BG], BF16, tag="bel")
                    nc.vector.tensor_tensor(out=belt[:], in0=ee[:],
                                            in1=bc_overy_small(rsm), op=OP.mult)
                    bel[g] = belt

            # ---------- epilogue: out = belief @ belief.T ----------
            for g in range(G):
                for bg in range(BG):
                    b = g * BG + bg
                    ps_b = psum.tile([Y, N], BF16, tag="ps_small", name="ps_b", bufs=1)
                    nc.tensor.transpose(ps_b[:], bel[g][:, :, bg], identity)
                    belT = smalls.tile([Y, N], BF16, tag="belT")
                    nc.scalar.copy(belT[:], ps_b[:])
                    ps_o = psum.tile([N, N], F32, tag="pso", bufs=1)
                    nc.tensor.matmul(ps_o[:], belT[:], belT[:])
                    ot = outp.tile([N, N], F32, tag="ot")
                    nc.scalar.copy(ot[:], ps_o[:])
                    nc.sync.dma_start(out=out_d[b, :, :], in_=ot[:])

    nc.finalize()
    return nc


def get_program():
    if "nc" not in _cache:
        _cache["nc"] = build_program()
    return _cache["nc"]


def make_in_maps(inp_data, unary_comp, binary_comp, affinity_mat):
    in_maps = []
    for i in range(NCORES):
        s = slice(i * BL, (i + 1) * BL)
        in_maps.append({
            "inp_data": np.ascontiguousarray(inp_data[s], np.float32),
            "unary_comp": np.ascontiguousarray(unary_comp[s], np.float32),
            "binary_comp": np.ascontiguousarray(binary_comp[s], np.float32),
            "affinity_mat": np.ascontiguousarray(affinity_mat[s], np.float32),
        })
    return in_maps


def run_bass(inp_data, unary_comp, binary_comp, affinity_mat, trace=False):
    from concourse.bass_utils import run_bass_kernel_spmd

    nc = get_program()
    in_maps = make_in_maps(inp_data, unary_comp, binary_comp, affinity_mat)
    res = run_bass_kernel_spmd(nc, in_maps, core_ids=list(range(NCORES)),
                               trace=trace)
    out = np.concatenate([np.asarray(res.results[i]["out"])
                          for i in range(NCORES)], axis=0)
    return out.astype(np.float32), res


def kernel(inp_data, unary_comp, binary_comp, affinity_mat,
           num_supports=80, lbp_count=8):
    assert int(num_supports) == NSUP and int(lbp_count) == ITERS + 1, (
        "kernel compiled for num_supports=80, lbp_count=8")
    inp_data = np.asarray(inp_data, np.float32)
    unary_comp = np.asarray(unary_comp, np.float32)
    binary_comp = np.asarray(binary_comp, np.float32)
    affinity_mat = np.asarray(affinity_mat, np.float32)
    out, _ = run_bass(inp_data, unary_comp, binary_comp, affinity_mat)
    return out
